# revision 40
# baseline (speedup 1.0000x reference)
"""Trainium2 Bass kernel for the Aligner2 problem (v3).

Computes, for each batch b:
  k = LReLU(conv3(LReLU(conv3(keys))))                   # [256, 520]
  q = LReLU(conv7(LReLU(conv7(LReLU(conv7(queries))))))  # [256, 2048]
  l[t,s] = SC*sum_c q[c,t] k[c,s] - TEMP*k2[s]   (q2 cancels in softmax)
  attn = exp(l)/z;  logp = l - ln(z);  z = sum_s exp(l)

v3 changes vs v2:
  - k2 is no longer rank-1-broadcast by the PE per score tile (was 25us of
    PE time); instead k2 is replicated across partitions once per batch via
    a ones[128,128] matmul, scaled to -TEMP*k2 (f16 k2s), and fused into
    the DVE pass: lraw = (sp*SC) + k2s via scalar_tensor_tensor. Exp then
    reads lraw (f16) instead of PSUM, so PSUM has a single reader (DVE).
  - score matmuls split 260+260 (two PSUM banks) instead of 512+8,
    removing the overhead-dominated 8-col tail matmuls.
  - qc1 uses a host-packed [5,128] (tap,chan) layout: 560 rows -> 2 DR
    pairs + 1 single fp8 matmul per (h, quarter) instead of 4 DR.
  - attn scaling moved to GpSimd; logp/lraw on DVE; exp/ln on Scalar.
  - back-to-back LDWEIGHTS with identical APs deduped before compile.
  - startup weight DMAs spread across engine queues.

Precision: query convs fp8e4m3 DoubleRow, key conv1 fp8 DR, key conv2 f16,
scores f16, outputs f16 (upcast to f32 on host).
Data-parallel over batch: 4 batches/core, 8 cores.
"""
import numpy as np

import concourse.bass as bass
import concourse.bacc as bacc
import concourse.tile as tile
from concourse import mybir
from concourse.bass_utils import run_bass_kernel_spmd

F32 = mybir.dt.float32
F16 = mybir.dt.float16
DT8 = mybir.dt.float8e4
AF = mybir.ActivationFunctionType
ALU = mybir.AluOpType
DR = mybir.MatmulPerfMode.DoubleRow

SLOPE = 0.3
TEMPERATURE = 0.0005
SC = 2.0 * TEMPERATURE

BPC = 4          # batches per core
N_CORES = 8
D_DEC, TQ = 80, 2048
D_ENC, TK = 512, 512
DH = 256
TK1 = TK + 4     # 516 after key conv1 (kernel 3, pad 3)
TK2 = TK + 8     # 520 after key conv2
HT1 = TK1 // 2   # 258
HT2 = TK2 // 2   # 260

import os
def _flag(name, default):
    return os.environ.get(name, default) == "1"

QK_FP8 = _flag("KV3_QK8", "0")      # score matmul fp8 DR (q3/ksb fp8 tiles)
QC1_F16 = _flag("KV3_QC1F16", "0")  # packed qc1 in f16 (5 singles) vs fp8
DEDUP_LDW = _flag("KV3_DEDUP", "1")
SPLIT_DMA = _flag("KV3_SPLITDMA", "1")

DT_Q = F16 if QC1_F16 else DT8    # qpk, wq1p
DT_S = DT8 if QK_FP8 else F16     # q3 / ksb score operands

ACT_SET_ALL = 6  # act_info.json set containing Prelu/Exp/Ln/Copy together


def build_program():
    nc = bacc.Bacc("TRN2", target_bir_lowering=False)

    # qc1 packed layout: row m = 80*j + c (tap j, channel c), m in [0,560),
    # split into 5 groups of 128; group g row r at col x holds
    # queries[c, x + j - 3] (zero outside), so a plain [:, g, x0:x0+512]
    # slice feeds tap j's shifted window directly.
    # All DRAM tensors are partition-major with contiguous per-partition
    # payloads so each is a single large-burst DMA.
    q_in = nc.dram_tensor("queries", [BPC, 128, 5, TQ + 8], DT_Q, kind="ExternalInput")
    k_in = nc.dram_tensor("keys", [BPC, 128, 4, TK + 6], DT8, kind="ExternalInput")
    # kc1 weights h-major so the first 98KB chunk alone starts the PE
    kw1t_d = nc.dram_tensor("kw1t", [128, 2, 4, 3, 128], DT8, kind="ExternalInput")
    kw2t_d = nc.dram_tensor("kw2t", [128, 2, 3, DH], F16, kind="ExternalInput")
    qw1t_d = nc.dram_tensor("qw1t", [128, 5, DH], DT_Q, kind="ExternalInput")
    qw2t_d = nc.dram_tensor("qw2t", [128, 2, 7, DH], DT8, kind="ExternalInput")
    qw3t_d = nc.dram_tensor("qw3t", [128, 2, 7, DH], DT8, kind="ExternalInput")
    # bias columns: (kb1 h0, kb1 h1, kb2 h0, kb2 h1, qb1.., qb2.., qb3..)
    bias_d = nc.dram_tensor("bias10", [128, 10], F32, kind="ExternalInput")
    # outputs grouped [g, p, i, s]: query t = 256*g + 128*i + p; contiguous
    # 2080B per partition per group DMA; host un-permutes.
    attn_out = nc.dram_tensor("attn_out", [BPC, 8, 128, 2, TK2], F16,
                              kind="ExternalOutput")
    logp_out = nc.dram_tensor("logp_out", [BPC, 8, 128, 2, TK2], F16,
                              kind="ExternalOutput")

    with tile.TileContext(nc) as tc:
        _emit(nc, tc, q_in, k_in, kw1t_d, kw2t_d, qw1t_d, qw2t_d, qw3t_d,
              bias_d, attn_out, logp_out)
    if DEDUP_LDW:
        _dedup_ldweights(nc)
    nc.compile()
    return nc


def _dedup_ldweights(nc):
    """Drop an InstLdweights whose weights AP is identical to the previous
    InstLdweights in the same block (nothing between them invalidates the
    PE-resident weights). Its waits/updates move to the following matmul."""
    for bb in nc.main_func.blocks:
        insts = bb.instructions
        last_key = None
        drop = []
        for idx, inst in enumerate(insts):
            if isinstance(inst, mybir.InstLdweights):
                key = str(inst.ins[0])
                if key == last_key:
                    drop.append(idx)
                last_key = key
        for idx in reversed(drop):
            inst = insts[idx]
            si = inst.sync_info
            if si is not None and (len(si.on_wait) or len(si.on_update)):
                nxt = insts[idx + 1]
                nsi = nxt.sync_info
                if nsi is None:
                    nxt.sync_info = si
                else:
                    nsi.on_wait.extend(si.on_wait)
                    nsi.on_update.extend(si.on_update)
            del insts[idx]


def _emit(nc, tc, q_in, k_in, kw1t_d, kw2t_d, qw1t_d, qw2t_d, qw3t_d,
          bias_d, attn_out, logp_out):
    from contextlib import ExitStack
    ctx = ExitStack()
    with ctx:
        singles = ctx.enter_context(tc.tile_pool(name="singles", bufs=1))
        p_in = ctx.enter_context(tc.tile_pool(name="p_in", bufs=2))
        p_k = ctx.enter_context(tc.tile_pool(name="p_k", bufs=2))
        p_q = ctx.enter_context(tc.tile_pool(name="p_q", bufs=2))
        p_soft = ctx.enter_context(tc.tile_pool(name="p_soft", bufs=6))
        p_small = ctx.enter_context(tc.tile_pool(name="p_small", bufs=8))
        p_att = ctx.enter_context(tc.tile_pool(name="p_att", bufs=3))
        p_lgp = ctx.enter_context(tc.tile_pool(name="p_lgp", bufs=3))
        pp_conv = ctx.enter_context(
            tc.tile_pool(name="pp_conv", bufs=2, space="PSUM"))
        pp_score = ctx.enter_context(
            tc.tile_pool(name="pp_score", bufs=2, space="PSUM"))

        nc.scalar.add_instruction(mybir.InstLoadActFuncSet(
            name=nc.get_next_instruction_name(), ins=[], outs=[],
            act_func_set_id=ACT_SET_ALL))

        # ---------------- weights / constants into SBUF (once) -------------
        # Startup transfers only start flowing after the ~8us NEFF preamble
        # and each queue sustains ~90-140 GB/s, so the 3.4MB of startup bytes
        # are choreographed across the three queues to meet each stage's
        # deadline: kc1 feed chunked (cp-outer kc1 starts on the first half),
        # qpk0 x-chunked so qc1's quarters stream in order.
        w_kw1 = singles.tile([128, 2, 4, 3, 128], DT8)
        kpad0 = p_in.tile([128, 4, TK + 6], DT8, tag="kpad")
        qpk0 = p_in.tile([128, 5, TQ + 8], DT_Q, tag="qpk")
        w_kw2 = singles.tile([128, 2, 3, DH], F16)
        w_qw1 = singles.tile([128, 5, DH], DT_Q)
        w_qw2 = singles.tile([128, 2, 7, DH], DT8)
        w_qw3 = singles.tile([128, 2, 7, DH], DT8)
        b_all = singles.tile([128, 10], F32)
        XC = (TQ + 8) // 4
        # sync: kc1 h0 weights (98KB chunks) + kpad first half, then the
        # h1 weights and qpk0 x-chunks 2,3
        nc.sync.dma_start(out=w_kw1[:, 0, 0:2], in_=kw1t_d[:, 0, 0:2])
        nc.sync.dma_start(out=kpad0[:, 0:2, :], in_=k_in[0, :, 0:2])
        nc.sync.dma_start(out=w_kw1[:, 0, 2:4], in_=kw1t_d[:, 0, 2:4])
        nc.sync.dma_start(out=w_kw1[:, 1], in_=kw1t_d[:, 1])
        nc.sync.dma_start(out=qpk0[:, :, 2 * XC:3 * XC],
                          in_=q_in[0, :, :, 2 * XC:3 * XC])
        nc.sync.dma_start(out=qpk0[:, :, 3 * XC:4 * XC],
                          in_=q_in[0, :, :, 3 * XC:4 * XC])
        # scalar: biases + qc1 weights, then qc2/qc3 weights
        nc.scalar.dma_start(out=b_all[:, :], in_=bias_d[:, :])
        nc.scalar.dma_start(out=w_qw1[:, :, :], in_=qw1t_d[:, :, :])
        nc.scalar.dma_start(out=w_qw2[:, :, :, :], in_=qw2t_d[:, :, :, :])
        nc.scalar.dma_start(out=w_qw3[:, :, :, :], in_=qw3t_d[:, :, :, :])
        # gpsimd: kpad second half, kc2 weights, qpk0 x-chunks 0,1
        nc.gpsimd.dma_start(out=kpad0[:, 2:4, :], in_=k_in[0, :, 2:4])
        nc.gpsimd.dma_start(out=w_kw2[:, :, :, :], in_=kw2t_d[:, :, :, :])
        nc.gpsimd.dma_start(out=qpk0[:, :, 0:XC], in_=q_in[0, :, :, 0:XC])
        nc.gpsimd.dma_start(out=qpk0[:, :, XC:2 * XC],
                            in_=q_in[0, :, :, XC:2 * XC])
        b_k1, b_k2 = b_all[:, 0:2], b_all[:, 2:4]
        b_q1, b_q2, b_q3 = b_all[:, 4:6], b_all[:, 6:8], b_all[:, 8:10]

        ones128 = singles.tile([128, 128], F16)
        nc.vector.memset(ones128, 1.0)

        # persistent padded intermediates; zero the margins once
        k1pad = singles.tile([128, 2, TK1 + 6], F16)
        q1pad = singles.tile([128, 2, TQ + 8], DT8)
        q2pad = singles.tile([128, 2, TQ + 8], DT8)
        for h in range(2):
            nc.vector.memset(k1pad[:, h, 0:3], 0.0)
            nc.vector.memset(k1pad[:, h, TK1 + 3:TK1 + 6], 0.0)
            nc.vector.memset(q1pad[:, h, 0:3], 0.0)
            nc.vector.memset(q1pad[:, h, TQ + 3:TQ + 8], 0.0)
            nc.vector.memset(q2pad[:, h, 0:3], 0.0)
            nc.vector.memset(q2pad[:, h, TQ + 3:TQ + 8], 0.0)

        # software pipeline: batch b's convs interleave with batch b-1's
        # score/softmax tiles so the in-order PE queue never head-blocks on
        # the softmax latency chain
        args = (nc, q_in, k_in, attn_out, logp_out,
                w_kw1, w_kw2, w_qw1, w_qw2, w_qw3,
                b_k1, b_k2, b_q1, b_q2, b_q3,
                ones128, k1pad, q1pad, q2pad,
                p_in, p_k, p_q, p_soft, p_small, p_att, p_lgp,
                pp_conv, pp_score)
        # Interleave: batch b's convs with batch b-1's score tiles (2 tiles
        # per conv step). The last batch computes qc3 quarter-by-quarter and
        # starts its own score tiles as each quarter's q3 lands, so the final
        # softmax drain overlaps the remaining conv work.
        prev_sc = None
        for b in range(BPC):
            last = b == BPC - 1
            out = {"kpad": kpad0, "qpk": qpk0} if b == 0 else {}
            cs = _conv_steps(b, out, last, *args)
            own_sc = None
            while True:
                try:
                    v = next(cs)
                except StopIteration:
                    break
                is_q3q = isinstance(v, tuple) and v[0] == "q3q"
                if last and is_q3q and own_sc is None:
                    own_sc = _score_steps(b, out, *args)
                if prev_sc is not None:
                    for _ in range(2):
                        try:
                            next(prev_sc)
                        except StopIteration:
                            prev_sc = None
                            break
                elif own_sc is not None and is_q3q:
                    for _ in range(v[1]):
                        try:
                            next(own_sc)
                        except StopIteration:
                            own_sc = None
                            break
            prev_sc = own_sc if last else _score_steps(b, out, *args)
        if prev_sc is not None:
            for _ in prev_sc:
                pass


def _conv_steps(b, out, last, nc, q_in, k_in, attn_out, logp_out,
                w_kw1, w_kw2, w_qw1, w_qw2, w_qw3,
                b_k1, b_k2, b_q1, b_q2, b_q3,
                ones128, k1pad, q1pad, q2pad,
                p_in, p_k, p_q, p_soft, p_small, p_att, p_lgp,
                pp_conv, pp_score):
    mm = nc.tensor.matmul
    act = nc.scalar.activation

    # ---------------- keys path ----------------
    if "kpad" in out:
        kpad = out["kpad"]
    else:
        kpad = p_in.tile([128, 4, TK + 6], DT8, tag="kpad")
        nc.sync.dma_start(out=kpad[:, :, :], in_=k_in[b])

    # key conv1: Cin=512, K=3, fp8 DR, out [256, 516] -> k1pad.
    # cp-outer so the first 6 matmuls only need the first half of the
    # kc1 weights/keys DMAs.
    for h in range(2):
        ps = pp_conv.tile([128, 2, 512], F32, tag="conv")
        for cp in range(2):
            for j in range(3):
                for th in range(2):
                    mm(ps[:, th, :HT1],
                       w_kw1[:, h, 2 * cp:2 * cp + 2, j, :],
                       kpad[:, 2 * cp:2 * cp + 2, HT1 * th + j:HT1 * th + j + HT1],
                       start=(cp == 0 and j == 0),
                       stop=(cp == 1 and j == 2), perf_mode=DR)
        act(k1pad[:, h, 3:3 + TK1].rearrange("p (a b) -> p a b", a=2),
            ps[:, :, :HT1], AF.Prelu, bias=b_k1[:, h:h + 1], scale=1.0,
            alpha=SLOPE)
        yield

    # key conv2: Cin=256, K=3, f16, out [256, 520] -> ksb
    ksb = p_k.tile([128, 2, TK2], DT_S, tag="ksb")
    ksb16 = ksb if not QK_FP8 else p_k.tile([128, 2, TK2], F16, tag="ksb16")
    for h in range(2):
        ps = pp_conv.tile([128, 2, 512], F32, tag="conv")
        for j in range(3):
            for c in range(2):
                for sh in range(2):
                    mm(ps[:, sh, :HT2],
                       w_kw2[:, c, j, 128 * h:128 * (h + 1)],
                       k1pad[:, c, HT2 * sh + j:HT2 * sh + j + HT2],
                       start=(j == 0 and c == 0),
                       stop=(j == 2 and c == 1))
        act(ksb16[:, h, :].rearrange("p (a b) -> p a b", a=2),
            ps[:, :, :HT2], AF.Prelu, bias=b_k2[:, h:h + 1], scale=1.0,
            alpha=SLOPE)
        if QK_FP8:
            act(ksb[:, h, :].rearrange("p (a b) -> p a b", a=2),
                ps[:, :, :HT2], AF.Prelu, bias=b_k2[:, h:h + 1], scale=1.0,
                alpha=SLOPE)
        yield

    # k2s[p, s] = -TEMP * sum_c k[c,s]^2, replicated over all partitions p:
    # ksq = ksb^2 on DVE, then ones[128,128].T @ ksq halves on the PE.
    ksq = p_k.tile([128, 2, TK2], F16, tag="ksq")
    nc.vector.tensor_tensor(out=ksq[:, :, :], in0=ksb16[:, :, :],
                            in1=ksb16[:, :, :], op=ALU.mult)
    psk2 = pp_score.tile([128, 2, 512], F32, tag="sc", name=f"k2rep{b}")
    for c in range(2):
        for sh in range(2):
            mm(psk2[:, sh, :HT2], ones128[:, :],
               ksq[:, c, HT2 * sh:HT2 * sh + HT2],
               start=(c == 0), stop=(c == 1))
    k2s = p_k.tile([128, 2, HT2], F16, tag="k2s")
    act(k2s[:, :, :], psk2[:, :, :HT2], AF.Copy, bias=0.0,
        scale=-float(TEMPERATURE))
    out["ksb"], out["k2s"] = ksb, k2s
    yield

    # ---------------- queries path ----------------
    if "qpk" in out:
        qpk = out["qpk"]
    else:
        qpk = p_in.tile([128, 5, TQ + 8], DT_Q, tag="qpk")
        nc.scalar.dma_start(out=qpk[:, 0:3, :], in_=q_in[b, :, 0:3])
        nc.gpsimd.dma_start(out=qpk[:, 3:5, :], in_=q_in[b, :, 3:5])

    # query conv1: packed 560 rows = (tap, chan); 2 DR pairs + 1 single.
    # g-outer so pss[0]'s prelu issues while pss[1] is still accumulating
    # (releases the PSUM slot early; no h-transition stall).
    for h in range(2):
        pss = [pp_conv.tile([128, 2, 512], F32, tag="conv", name=f"q1_{b}_{h}_{g}")
               for g in range(2)]
        if QC1_F16:
            grps = [(g, None) for g in range(5)]
        else:
            grps = [(0, DR), (2, DR), (4, None)]
        for g in range(2):
            for gi, (g0, pm) in enumerate(grps):
                first, last = gi == 0, gi == len(grps) - 1
                for i in range(2):
                    t4 = 2 * g + i
                    if pm is DR:
                        mm(pss[g][:, i, :],
                           w_qw1[:, g0:g0 + 2, 128 * h:128 * (h + 1)],
                           qpk[:, g0:g0 + 2, 512 * t4:512 * t4 + 512],
                           start=first, stop=last, perf_mode=DR)
                    else:
                        mm(pss[g][:, i, :],
                           w_qw1[:, g0, 128 * h:128 * (h + 1)],
                           qpk[:, g0, 512 * t4:512 * t4 + 512],
                           start=first, stop=last)
            act(q1pad[:, h, 3 + 1024 * g:3 + 1024 * (g + 1)]
                .rearrange("p (a b) -> p a b", a=2), pss[g][:, :, :],
                AF.Prelu, bias=b_q1[:, h:h + 1], scale=1.0, alpha=SLOPE)
        yield

    # query conv2: Cin=256, K=7, channel-pair DoubleRow, g-outer
    for h in range(2):
        pss = [pp_conv.tile([128, 2, 512], F32, tag="conv",
                            name=f"q2_{b}_{h}_{g}") for g in range(2)]
        for g in range(2):
            for j in range(7):
                for i in range(2):
                    t4 = 2 * g + i
                    mm(pss[g][:, i, :],
                       w_qw2[:, 0:2, j, 128 * h:128 * (h + 1)],
                       q1pad[:, 0:2, 512 * t4 + j:512 * t4 + j + 512],
                       start=(j == 0), stop=(j == 6), perf_mode=DR)
            act(q2pad[:, h, 3 + 1024 * g:3 + 1024 * (g + 1)]
                .rearrange("p (a b) -> p a b", a=2), pss[g][:, :, :],
                AF.Prelu, bias=b_q2[:, h:h + 1], scale=1.0, alpha=SLOPE)
        yield

    # query conv3 -> q3 (score operand dtype)
    q3 = p_q.tile([128, 2, TQ], DT_S, tag="q3")
    out["q3"] = q3
    if last:
        # quarter-wise (both h per quarter) so score tiles 4q..4q+3 can
        # start as soon as quarter q lands; driver interleaves on "q3q"
        for t4 in range(4):
            psq = pp_conv.tile([128, 2, 512], F32, tag="conv",
                               name=f"q3_{b}_q{t4}")
            for h in range(2):
                for j in range(7):
                    mm(psq[:, h, :],
                       w_qw3[:, 0:2, j, 128 * h:128 * (h + 1)],
                       q2pad[:, 0:2, 512 * t4 + j:512 * t4 + j + 512],
                       start=(j == 0), stop=(j == 6), perf_mode=DR)
                act(q3[:, h, 512 * t4:512 * (t4 + 1)], psq[:, h, :],
                    AF.Prelu, bias=b_q3[:, h:h + 1], scale=1.0, alpha=SLOPE)
            yield ("q3q", 4)
    else:
        for h in range(2):
            pss = [pp_conv.tile([128, 2, 512], F32, tag="conv",
                                name=f"q3_{b}_{h}_{g}") for g in range(2)]
            for g in range(2):
                for j in range(7):
                    for i in range(2):
                        t4 = 2 * g + i
                        mm(pss[g][:, i, :],
                           w_qw3[:, 0:2, j, 128 * h:128 * (h + 1)],
                           q2pad[:, 0:2, 512 * t4 + j:512 * t4 + j + 512],
                           start=(j == 0), stop=(j == 6), perf_mode=DR)
                act(q3[:, h, 1024 * g:1024 * (g + 1)]
                    .rearrange("p (a b) -> p a b", a=2), pss[g][:, :, :],
                    AF.Prelu, bias=b_q3[:, h:h + 1], scale=1.0, alpha=SLOPE)
            yield


def _score_steps(b, out, nc, q_in, k_in, attn_out, logp_out,
                 w_kw1, w_kw2, w_qw1, w_qw2, w_qw3,
                 b_k1, b_k2, b_q1, b_q2, b_q3,
                 ones128, k1pad, q1pad, q2pad,
                 p_in, p_k, p_q, p_soft, p_small, p_att, p_lgp,
                 pp_conv, pp_score):
    mm = nc.tensor.matmul
    act = nc.scalar.activation
    ksb, k2s, q3 = out["ksb"], out["k2s"], out["q3"]
    attn_g = logp_g = None
    for t in range(TQ // 128):
        g, i = divmod(t, 2)
        # the last 4 tiles borrow the (then-idle) conv PSUM pool: 4 score
        # tiles in flight deepens the final drain pipeline
        pool = pp_conv if (b == BPC - 1 and t >= 12) else pp_score
        sp = pool.tile([128, 2, 512], F32,
                       tag="sc" if pool is pp_score else "conv",
                       name=f"sp{b}_{t}")
        if QK_FP8:
            for sh in range(2):
                mm(sp[:, sh, :HT2], q3[:, :, 128 * t:128 * (t + 1)],
                   ksb[:, :, HT2 * sh:HT2 * sh + HT2],
                   start=True, stop=True, perf_mode=DR)
        else:
            for c in range(2):
                q3w = q3[:, c, 128 * t:128 * (t + 1)]
                for sh in range(2):
                    mm(sp[:, sh, :HT2], q3w,
                       ksb[:, c, HT2 * sh:HT2 * sh + HT2],
                       start=(c == 0), stop=(c == 1))

        # lraw = SC*sp + k2s  (single PSUM reader; frees the bank)
        lraw = p_soft.tile([128, 2, HT2], F16, tag="lraw", name=f"lr{b}_{t}")
        nc.vector.scalar_tensor_tensor(
            out=lraw[:, :, :], in0=sp[:, :, :HT2], scalar=float(SC),
            in1=k2s[:, :, :], op0=ALU.mult, op1=ALU.add)

        esb = p_soft.tile([128, 2, HT2], F16, tag="esb", name=f"esb{b}_{t}")
        z = p_small.tile([128, 1], F32, tag="z")
        act(esb[:, :, :], lraw[:, :, :], AF.Exp, bias=0.0, scale=1.0,
            accum_out=z)
        if i == 0:
            attn_g = p_att.tile([128, 2, TK2], F16, tag="attn")
            logp_g = p_lgp.tile([128, 2, TK2], F16, tag="logp")
        lnz = p_small.tile([128, 1], F32, tag="lnz")
        act(lnz, z, AF.Ln)
        rz = p_small.tile([128, 1], F32, tag="rz")
        nc.vector.reciprocal(rz, z)
        nc.vector.tensor_scalar(
            attn_g[:, i, :], esb.rearrange("p a b -> p (a b)"),
            rz, None, ALU.mult)
        nc.vector.tensor_scalar(
            logp_g[:, i, :], lraw.rearrange("p a b -> p (a b)"),
            lnz, None, ALU.subtract)
        if b == BPC - 1 and g == 7:
            # last group: per-tile DMA so the final transfer starts early
            nc.sync.dma_start(out=attn_out[b, g, :, i], in_=attn_g[:, i, :])
            nc.sync.dma_start(out=logp_out[b, g, :, i], in_=logp_g[:, i, :])
        elif i == 1:
            nc.sync.dma_start(out=attn_out[b, g], in_=attn_g[:, :, :])
            nc.sync.dma_start(out=logp_out[b, g], in_=logp_g[:, :, :])
        yield


_PROGRAM = None


def _get_program():
    global _PROGRAM
    if _PROGRAM is None:
        _PROGRAM = build_program()
    return _PROGRAM


def prep_inputs(queries, keys, kw1, kb1, kw2, kb2, qw1, qb1, qw2, qb2, qw3, qb3):
    """Build the 8 per-core input maps from full-size inputs.

    All tensors are laid out partition-major (first dim = SBUF partition)
    with contiguous per-partition payloads -> one large-burst DMA each."""
    f32 = np.float32
    n_8 = mybir.dt.np(DT8)
    n_q = mybir.dt.np(DT_Q)

    # [128p, 2h, 4cgrp, 3tap, 128out]
    kw1t = np.ascontiguousarray(
        np.transpose(kw1, (1, 2, 0)).reshape(4, 128, 3, 2, 128)
        .transpose(1, 3, 0, 2, 4).astype(n_8))
    kw2t = np.ascontiguousarray(np.transpose(
        np.transpose(kw2, (1, 2, 0)).reshape(2, 128, 3, DH), (1, 0, 2, 3))
        .astype(np.float16))
    # qc1 packed weights: row m = 80*j + c -> qw1[o, c, j]
    qw1t = np.zeros((5 * 128, DH), n_q)
    qw1t[:560] = np.transpose(qw1, (2, 1, 0)).reshape(560, DH)
    qw1t = np.ascontiguousarray(
        np.transpose(qw1t.reshape(5, 128, DH), (1, 0, 2)))
    qw2t = np.ascontiguousarray(np.transpose(
        np.transpose(qw2, (1, 2, 0)).reshape(2, 128, 7, DH), (1, 0, 2, 3))
        .astype(n_8))
    qw3t = np.ascontiguousarray(np.transpose(
        np.transpose(qw3, (1, 2, 0)).reshape(2, 128, 7, DH), (1, 0, 2, 3))
        .astype(n_8))
    bias10 = np.ascontiguousarray(np.stack(
        [b.reshape(2, 128) for b in (kb1, kb2, qb1, qb2, qb3)],
        axis=0).reshape(10, 128).T.astype(f32))
    shared = dict(kw1t=kw1t, kw2t=kw2t, qw1t=qw1t, qw2t=qw2t, qw3t=qw3t,
                  bias10=bias10)
    B = queries.shape[0]
    # packed queries: plane m = (j, c): qp[b, m, x] = queries[b, c, x + j - 3]
    qp = np.zeros((B, 5 * 128, TQ + 8), n_q)
    qd = queries.astype(n_q)
    for j in range(7):
        lo, hi = max(0, 3 - j), min(TQ, TQ + 3 - j)
        qp[:, 80 * j:80 * (j + 1), lo:hi] = qd[:, :, lo + j - 3:hi + j - 3]
    qp = np.transpose(qp.reshape(B, 5, 128, TQ + 8), (0, 2, 1, 3))
    kp = np.zeros((B, D_ENC, TK + 6), n_8)
    kp[:, :, 3:TK + 3] = keys.astype(n_8)
    kp = np.transpose(kp.reshape(B, 4, 128, TK + 6), (0, 2, 1, 3))
    in_maps = []
    for i in range(N_CORES):
        m = dict(shared)
        m["queries"] = np.ascontiguousarray(qp[BPC * i:BPC * (i + 1)])
        m["keys"] = np.ascontiguousarray(kp[BPC * i:BPC * (i + 1)])
        in_maps.append(m)
    return in_maps


def run(in_maps, **kwargs):
    nc = _get_program()
    return run_bass_kernel_spmd(nc, in_maps, core_ids=list(range(N_CORES)), **kwargs)


def kernel(queries, keys, kw1, kb1, kw2, kb2, qw1, qb1, qw2, qb2, qw3, qb3,
           **kwargs):
    in_maps = prep_inputs(queries, keys, kw1, kb1, kw2, kb2,
                          qw1, qb1, qw2, qb2, qw3, qb3)
    res = run(in_maps)
    # device layout [b, g, p, i, s] with t = 256*g + 128*i + p
    attn = np.concatenate([np.asarray(r["attn_out"], np.float32)
                           for r in res.results], axis=0)
    logp = np.concatenate([np.asarray(r["logp_out"], np.float32)
                           for r in res.results], axis=0)
    B = attn.shape[0]
    attn = np.transpose(attn, (0, 1, 3, 2, 4)).reshape(B, 1, TQ, TK2)
    logp = np.transpose(logp, (0, 1, 3, 2, 4)).reshape(B, 1, TQ, TK2)
    return attn, logp


# revision 41
# speedup vs baseline: 1.2107x; 1.2107x over previous
"""Trainium2 Bass kernel for the Aligner2 problem (v3).

Computes, for each batch b:
  k = LReLU(conv3(LReLU(conv3(keys))))                   # [256, 520]
  q = LReLU(conv7(LReLU(conv7(LReLU(conv7(queries))))))  # [256, 2048]
  l[t,s] = SC*sum_c q[c,t] k[c,s] - TEMP*k2[s]   (q2 cancels in softmax)
  attn = exp(l)/z;  logp = l - ln(z);  z = sum_s exp(l)

v3 changes vs v2:
  - k2 is no longer rank-1-broadcast by the PE per score tile (was 25us of
    PE time); instead k2 is replicated across partitions once per batch via
    a ones[128,128] matmul, scaled to -TEMP*k2 (f16 k2s), and fused into
    the DVE pass: lraw = (sp*SC) + k2s via scalar_tensor_tensor. Exp then
    reads lraw (f16) instead of PSUM, so PSUM has a single reader (DVE).
  - score matmuls split 260+260 (two PSUM banks) instead of 512+8,
    removing the overhead-dominated 8-col tail matmuls.
  - qc1 uses a host-packed [5,128] (tap,chan) layout: 560 rows -> 2 DR
    pairs + 1 single fp8 matmul per (h, quarter) instead of 4 DR.
  - attn scaling moved to GpSimd; logp/lraw on DVE; exp/ln on Scalar.
  - back-to-back LDWEIGHTS with identical APs deduped before compile.
  - startup weight DMAs spread across engine queues.

Precision: query convs fp8e4m3 DoubleRow, key conv1 fp8 DR, key conv2 f16,
scores f16, outputs f16 (upcast to f32 on host).
Data-parallel over batch: 4 batches/core, 8 cores.
"""
import numpy as np

import concourse.bass as bass
import concourse.bacc as bacc
import concourse.tile as tile
from concourse import mybir
from concourse.bass_utils import run_bass_kernel_spmd

F32 = mybir.dt.float32
F16 = mybir.dt.float16
DT8 = mybir.dt.float8e4
AF = mybir.ActivationFunctionType
ALU = mybir.AluOpType
DR = mybir.MatmulPerfMode.DoubleRow

SLOPE = 0.3
TEMPERATURE = 0.0005
SC = 2.0 * TEMPERATURE

BPC = 4          # batches per core
N_CORES = 8
D_DEC, TQ = 80, 2048
D_ENC, TK = 512, 512
DH = 256
TK1 = TK + 4     # 516 after key conv1 (kernel 3, pad 3)
TK2 = TK + 8     # 520 after key conv2
HT1 = TK1 // 2   # 258
HT2 = TK2 // 2   # 260

import os
def _flag(name, default):
    return os.environ.get(name, default) == "1"

QK_FP8 = _flag("KV3_QK8", "0")      # score matmul fp8 DR (q3/ksb fp8 tiles)
QC1_F16 = _flag("KV3_QC1F16", "0")  # packed qc1 in f16 (5 singles) vs fp8
DEDUP_LDW = _flag("KV3_DEDUP", "1")
SPLIT_DMA = _flag("KV3_SPLITDMA", "1")

DT_Q = F16 if QC1_F16 else DT8    # qpk, wq1p
DT_S = DT8 if QK_FP8 else F16     # q3 / ksb score operands

ACT_SET_ALL = 6  # act_info.json set containing Prelu/Exp/Ln/Copy together


def build_program():
    nc = bacc.Bacc("TRN2", target_bir_lowering=False)

    # qc1 packed layout: row m = 80*j + c (tap j, channel c), m in [0,560),
    # split into 5 groups of 128; group g row r at col x holds
    # queries[c, x + j - 3] (zero outside), so a plain [:, g, x0:x0+512]
    # slice feeds tap j's shifted window directly.
    # All DRAM tensors are partition-major with contiguous per-partition
    # payloads so each is a single large-burst DMA.
    q_in = nc.dram_tensor("queries", [BPC, 128, 5, TQ + 8], DT_Q, kind="ExternalInput")
    k_in = nc.dram_tensor("keys", [BPC, 128, 4, TK + 6], DT8, kind="ExternalInput")
    kw1t_d = nc.dram_tensor("kw1t", [128, 4, 3, DH], DT8, kind="ExternalInput")
    kw2t_d = nc.dram_tensor("kw2t", [128, 2, 3, DH], F16, kind="ExternalInput")
    qw1t_d = nc.dram_tensor("qw1t", [128, 5, DH], DT_Q, kind="ExternalInput")
    qw2t_d = nc.dram_tensor("qw2t", [128, 2, 7, DH], DT8, kind="ExternalInput")
    qw3t_d = nc.dram_tensor("qw3t", [128, 2, 7, DH], DT8, kind="ExternalInput")
    # bias columns: (kb1 h0, kb1 h1, kb2 h0, kb2 h1, qb1.., qb2.., qb3..)
    bias_d = nc.dram_tensor("bias10", [128, 10], F32, kind="ExternalInput")
    # outputs grouped [g, p, i, s]: query t = 256*g + 128*i + p; contiguous
    # 2080B per partition per group DMA; host un-permutes.
    attn_out = nc.dram_tensor("attn_out", [BPC, 8, 128, 2, TK2], F16,
                              kind="ExternalOutput")
    logp_out = nc.dram_tensor("logp_out", [BPC, 8, 128, 2, TK2], F16,
                              kind="ExternalOutput")

    with tile.TileContext(nc) as tc:
        _emit(nc, tc, q_in, k_in, kw1t_d, kw2t_d, qw1t_d, qw2t_d, qw3t_d,
              bias_d, attn_out, logp_out)
    if DEDUP_LDW:
        _dedup_ldweights(nc)
    nc.compile()
    return nc


def _dedup_ldweights(nc):
    """Drop an InstLdweights whose weights AP is identical to the previous
    InstLdweights in the same block (nothing between them invalidates the
    PE-resident weights). Its waits/updates move to the following matmul."""
    for bb in nc.main_func.blocks:
        insts = bb.instructions
        last_key = None
        drop = []
        for idx, inst in enumerate(insts):
            if isinstance(inst, mybir.InstLdweights):
                key = str(inst.ins[0])
                if key == last_key:
                    drop.append(idx)
                last_key = key
        for idx in reversed(drop):
            inst = insts[idx]
            si = inst.sync_info
            if si is not None and (len(si.on_wait) or len(si.on_update)):
                nxt = insts[idx + 1]
                nsi = nxt.sync_info
                if nsi is None:
                    nxt.sync_info = si
                else:
                    nsi.on_wait.extend(si.on_wait)
                    nsi.on_update.extend(si.on_update)
            del insts[idx]


def _emit(nc, tc, q_in, k_in, kw1t_d, kw2t_d, qw1t_d, qw2t_d, qw3t_d,
          bias_d, attn_out, logp_out):
    from contextlib import ExitStack
    ctx = ExitStack()
    with ctx:
        singles = ctx.enter_context(tc.tile_pool(name="singles", bufs=1))
        p_in = ctx.enter_context(tc.tile_pool(name="p_in", bufs=2))
        p_k = ctx.enter_context(tc.tile_pool(name="p_k", bufs=2))
        p_q = ctx.enter_context(tc.tile_pool(name="p_q", bufs=2))
        p_soft = ctx.enter_context(tc.tile_pool(name="p_soft", bufs=6))
        p_small = ctx.enter_context(tc.tile_pool(name="p_small", bufs=8))
        p_att = ctx.enter_context(tc.tile_pool(name="p_att", bufs=3))
        p_lgp = ctx.enter_context(tc.tile_pool(name="p_lgp", bufs=3))
        pp_conv = ctx.enter_context(
            tc.tile_pool(name="pp_conv", bufs=2, space="PSUM"))
        pp_score = ctx.enter_context(
            tc.tile_pool(name="pp_score", bufs=2, space="PSUM"))

        nc.scalar.add_instruction(mybir.InstLoadActFuncSet(
            name=nc.get_next_instruction_name(), ins=[], outs=[],
            act_func_set_id=ACT_SET_ALL))

        # ---------------- weights / constants into SBUF (once) -------------
        # Startup transfers only start flowing after the ~8us NEFF preamble
        # and each queue sustains ~90-140 GB/s, so the 3.4MB of startup bytes
        # are choreographed across the three queues to meet each stage's
        # deadline: kc1 feed chunked (cp-outer kc1 starts on the first half),
        # qpk0 x-chunked so qc1's quarters stream in order.
        w_kw1 = singles.tile([128, 4, 3, DH], DT8)
        kpad0 = p_in.tile([128, 4, TK + 6], DT8, tag="kpad")
        qpk0 = p_in.tile([128, 5, TQ + 8], DT_Q, tag="qpk")
        w_kw2 = singles.tile([128, 2, 3, DH], F16)
        w_qw1 = singles.tile([128, 5, DH], DT_Q)
        w_qw2 = singles.tile([128, 2, 7, DH], DT8)
        w_qw3 = singles.tile([128, 2, 7, DH], DT8)
        b_all = singles.tile([128, 10], F32)
        XC = (TQ + 8) // 4
        # sync: kc1 weights + kpad first half, then qpk0 x-chunks 2,3
        nc.sync.dma_start(out=w_kw1[:, 0:2], in_=kw1t_d[:, 0:2])
        nc.sync.dma_start(out=kpad0[:, 0:2, :], in_=k_in[0, :, 0:2])
        nc.sync.dma_start(out=w_kw1[:, 2:4], in_=kw1t_d[:, 2:4])
        nc.sync.dma_start(out=qpk0[:, :, 2 * XC:3 * XC],
                          in_=q_in[0, :, :, 2 * XC:3 * XC])
        nc.sync.dma_start(out=qpk0[:, :, 3 * XC:4 * XC],
                          in_=q_in[0, :, :, 3 * XC:4 * XC])
        # scalar: biases + qc1 weights, then qc2/qc3 weights
        nc.scalar.dma_start(out=b_all[:, :], in_=bias_d[:, :])
        nc.scalar.dma_start(out=w_qw1[:, :, :], in_=qw1t_d[:, :, :])
        nc.scalar.dma_start(out=w_qw2[:, :, :, :], in_=qw2t_d[:, :, :, :])
        nc.scalar.dma_start(out=w_qw3[:, :, :, :], in_=qw3t_d[:, :, :, :])
        # gpsimd: kpad second half, kc2 weights, qpk0 x-chunks 0,1
        nc.gpsimd.dma_start(out=kpad0[:, 2:4, :], in_=k_in[0, :, 2:4])
        nc.gpsimd.dma_start(out=w_kw2[:, :, :, :], in_=kw2t_d[:, :, :, :])
        nc.gpsimd.dma_start(out=qpk0[:, :, 0:XC], in_=q_in[0, :, :, 0:XC])
        nc.gpsimd.dma_start(out=qpk0[:, :, XC:2 * XC],
                            in_=q_in[0, :, :, XC:2 * XC])
        b_k1, b_k2 = b_all[:, 0:2], b_all[:, 2:4]
        b_q1, b_q2, b_q3 = b_all[:, 4:6], b_all[:, 6:8], b_all[:, 8:10]

        ones128 = singles.tile([128, 128], F16)
        nc.vector.memset(ones128, 1.0)

        # persistent padded intermediates; zero the margins once
        k1pad = singles.tile([128, 2, TK1 + 6], F16)
        q1pad = singles.tile([128, 2, TQ + 8], DT8)
        q2pad = singles.tile([128, 2, TQ + 8], DT8)
        for h in range(2):
            nc.vector.memset(k1pad[:, h, 0:3], 0.0)
            nc.vector.memset(k1pad[:, h, TK1 + 3:TK1 + 6], 0.0)
            nc.vector.memset(q1pad[:, h, 0:3], 0.0)
            nc.vector.memset(q1pad[:, h, TQ + 3:TQ + 8], 0.0)
            nc.vector.memset(q2pad[:, h, 0:3], 0.0)
            nc.vector.memset(q2pad[:, h, TQ + 3:TQ + 8], 0.0)

        # software pipeline: batch b's convs interleave with batch b-1's
        # score/softmax tiles so the in-order PE queue never head-blocks on
        # the softmax latency chain
        args = (nc, q_in, k_in, attn_out, logp_out,
                w_kw1, w_kw2, w_qw1, w_qw2, w_qw3,
                b_k1, b_k2, b_q1, b_q2, b_q3,
                ones128, k1pad, q1pad, q2pad,
                p_in, p_k, p_q, p_soft, p_small, p_att, p_lgp,
                pp_conv, pp_score)
        # Interleave: batch b's convs with batch b-1's score tiles (2 tiles
        # per conv step). The last batch computes qc3 quarter-by-quarter and
        # starts its own score tiles as each quarter's q3 lands, so the final
        # softmax drain overlaps the remaining conv work.
        prev_sc = None
        for b in range(BPC):
            last = b == BPC - 1
            out = {"kpad": kpad0, "qpk": qpk0} if b == 0 else {}
            cs = _conv_steps(b, out, last, *args)
            own_sc = None
            while True:
                try:
                    v = next(cs)
                except StopIteration:
                    break
                is_q3q = isinstance(v, tuple) and v[0] == "q3q"
                if last and is_q3q and own_sc is None:
                    own_sc = _score_steps(b, out, *args)
                if prev_sc is not None:
                    for _ in range(2):
                        try:
                            next(prev_sc)
                        except StopIteration:
                            prev_sc = None
                            break
                elif own_sc is not None and is_q3q:
                    for _ in range(v[1]):
                        try:
                            next(own_sc)
                        except StopIteration:
                            own_sc = None
                            break
            prev_sc = own_sc if last else _score_steps(b, out, *args)
        if prev_sc is not None:
            for _ in prev_sc:
                pass


def _conv_steps(b, out, last, nc, q_in, k_in, attn_out, logp_out,
                w_kw1, w_kw2, w_qw1, w_qw2, w_qw3,
                b_k1, b_k2, b_q1, b_q2, b_q3,
                ones128, k1pad, q1pad, q2pad,
                p_in, p_k, p_q, p_soft, p_small, p_att, p_lgp,
                pp_conv, pp_score):
    mm = nc.tensor.matmul
    act = nc.scalar.activation

    # ---------------- keys path ----------------
    if "kpad" in out:
        kpad = out["kpad"]
    else:
        kpad = p_in.tile([128, 4, TK + 6], DT8, tag="kpad")
        nc.sync.dma_start(out=kpad[:, :, :], in_=k_in[b])

    # key conv1: Cin=512, K=3, fp8 DR, out [256, 516] -> k1pad.
    # cp-outer so the first 6 matmuls only need the first half of the
    # kc1 weights/keys DMAs.
    for h in range(2):
        ps = pp_conv.tile([128, 2, 512], F32, tag="conv")
        for cp in range(2):
            for j in range(3):
                for th in range(2):
                    mm(ps[:, th, :HT1],
                       w_kw1[:, 2 * cp:2 * cp + 2, j, 128 * h:128 * (h + 1)],
                       kpad[:, 2 * cp:2 * cp + 2, HT1 * th + j:HT1 * th + j + HT1],
                       start=(cp == 0 and j == 0),
                       stop=(cp == 1 and j == 2), perf_mode=DR)
        act(k1pad[:, h, 3:3 + TK1].rearrange("p (a b) -> p a b", a=2),
            ps[:, :, :HT1], AF.Prelu, bias=b_k1[:, h:h + 1], scale=1.0,
            alpha=SLOPE)
        yield

    # key conv2: Cin=256, K=3, f16, out [256, 520] -> ksb
    ksb = p_k.tile([128, 2, TK2], DT_S, tag="ksb")
    ksb16 = ksb if not QK_FP8 else p_k.tile([128, 2, TK2], F16, tag="ksb16")
    for h in range(2):
        ps = pp_conv.tile([128, 2, 512], F32, tag="conv")
        for j in range(3):
            for c in range(2):
                for sh in range(2):
                    mm(ps[:, sh, :HT2],
                       w_kw2[:, c, j, 128 * h:128 * (h + 1)],
                       k1pad[:, c, HT2 * sh + j:HT2 * sh + j + HT2],
                       start=(j == 0 and c == 0),
                       stop=(j == 2 and c == 1))
        act(ksb16[:, h, :].rearrange("p (a b) -> p a b", a=2),
            ps[:, :, :HT2], AF.Prelu, bias=b_k2[:, h:h + 1], scale=1.0,
            alpha=SLOPE)
        if QK_FP8:
            act(ksb[:, h, :].rearrange("p (a b) -> p a b", a=2),
                ps[:, :, :HT2], AF.Prelu, bias=b_k2[:, h:h + 1], scale=1.0,
                alpha=SLOPE)
        yield

    # k2s[p, s] = -TEMP * sum_c k[c,s]^2, replicated over all partitions p:
    # ksq = ksb^2 on DVE, then ones[128,128].T @ ksq halves on the PE.
    ksq = p_k.tile([128, 2, TK2], F16, tag="ksq")
    nc.vector.tensor_tensor(out=ksq[:, :, :], in0=ksb16[:, :, :],
                            in1=ksb16[:, :, :], op=ALU.mult)
    psk2 = pp_score.tile([128, 2, 512], F32, tag="sc", name=f"k2rep{b}")
    for c in range(2):
        for sh in range(2):
            mm(psk2[:, sh, :HT2], ones128[:, :],
               ksq[:, c, HT2 * sh:HT2 * sh + HT2],
               start=(c == 0), stop=(c == 1))
    k2s = p_k.tile([128, 2, HT2], F16, tag="k2s")
    act(k2s[:, :, :], psk2[:, :, :HT2], AF.Copy, bias=0.0,
        scale=-float(TEMPERATURE))
    out["ksb"], out["k2s"] = ksb, k2s
    yield

    # ---------------- queries path ----------------
    if "qpk" in out:
        qpk = out["qpk"]
    else:
        qpk = p_in.tile([128, 5, TQ + 8], DT_Q, tag="qpk")
        nc.scalar.dma_start(out=qpk[:, 0:3, :], in_=q_in[b, :, 0:3])
        nc.gpsimd.dma_start(out=qpk[:, 3:5, :], in_=q_in[b, :, 3:5])

    # query conv1: packed 560 rows = (tap, chan); 2 DR pairs + 1 single.
    # g-outer so pss[0]'s prelu issues while pss[1] is still accumulating
    # (releases the PSUM slot early; no h-transition stall).
    for h in range(2):
        pss = [pp_conv.tile([128, 2, 512], F32, tag="conv", name=f"q1_{b}_{h}_{g}")
               for g in range(2)]
        if QC1_F16:
            grps = [(g, None) for g in range(5)]
        else:
            grps = [(0, DR), (2, DR), (4, None)]
        for g in range(2):
            for gi, (g0, pm) in enumerate(grps):
                first, last = gi == 0, gi == len(grps) - 1
                for i in range(2):
                    t4 = 2 * g + i
                    if pm is DR:
                        mm(pss[g][:, i, :],
                           w_qw1[:, g0:g0 + 2, 128 * h:128 * (h + 1)],
                           qpk[:, g0:g0 + 2, 512 * t4:512 * t4 + 512],
                           start=first, stop=last, perf_mode=DR)
                    else:
                        mm(pss[g][:, i, :],
                           w_qw1[:, g0, 128 * h:128 * (h + 1)],
                           qpk[:, g0, 512 * t4:512 * t4 + 512],
                           start=first, stop=last)
            act(q1pad[:, h, 3 + 1024 * g:3 + 1024 * (g + 1)]
                .rearrange("p (a b) -> p a b", a=2), pss[g][:, :, :],
                AF.Prelu, bias=b_q1[:, h:h + 1], scale=1.0, alpha=SLOPE)
        yield

    # query conv2: Cin=256, K=7, channel-pair DoubleRow, g-outer
    for h in range(2):
        pss = [pp_conv.tile([128, 2, 512], F32, tag="conv",
                            name=f"q2_{b}_{h}_{g}") for g in range(2)]
        for g in range(2):
            for j in range(7):
                for i in range(2):
                    t4 = 2 * g + i
                    mm(pss[g][:, i, :],
                       w_qw2[:, 0:2, j, 128 * h:128 * (h + 1)],
                       q1pad[:, 0:2, 512 * t4 + j:512 * t4 + j + 512],
                       start=(j == 0), stop=(j == 6), perf_mode=DR)
            act(q2pad[:, h, 3 + 1024 * g:3 + 1024 * (g + 1)]
                .rearrange("p (a b) -> p a b", a=2), pss[g][:, :, :],
                AF.Prelu, bias=b_q2[:, h:h + 1], scale=1.0, alpha=SLOPE)
        yield

    # query conv3 -> q3 (score operand dtype)
    q3 = p_q.tile([128, 2, TQ], DT_S, tag="q3")
    out["q3"] = q3
    if last:
        # quarter-wise (both h per quarter) so score tiles 4q..4q+3 can
        # start as soon as quarter q lands; driver interleaves on "q3q"
        for t4 in range(4):
            psq = pp_conv.tile([128, 2, 512], F32, tag="conv",
                               name=f"q3_{b}_q{t4}")
            for h in range(2):
                for j in range(7):
                    mm(psq[:, h, :],
                       w_qw3[:, 0:2, j, 128 * h:128 * (h + 1)],
                       q2pad[:, 0:2, 512 * t4 + j:512 * t4 + j + 512],
                       start=(j == 0), stop=(j == 6), perf_mode=DR)
                act(q3[:, h, 512 * t4:512 * (t4 + 1)], psq[:, h, :],
                    AF.Prelu, bias=b_q3[:, h:h + 1], scale=1.0, alpha=SLOPE)
            yield ("q3q", 4)
    else:
        for h in range(2):
            pss = [pp_conv.tile([128, 2, 512], F32, tag="conv",
                                name=f"q3_{b}_{h}_{g}") for g in range(2)]
            for g in range(2):
                for j in range(7):
                    for i in range(2):
                        t4 = 2 * g + i
                        mm(pss[g][:, i, :],
                           w_qw3[:, 0:2, j, 128 * h:128 * (h + 1)],
                           q2pad[:, 0:2, 512 * t4 + j:512 * t4 + j + 512],
                           start=(j == 0), stop=(j == 6), perf_mode=DR)
                act(q3[:, h, 1024 * g:1024 * (g + 1)]
                    .rearrange("p (a b) -> p a b", a=2), pss[g][:, :, :],
                    AF.Prelu, bias=b_q3[:, h:h + 1], scale=1.0, alpha=SLOPE)
            yield


def _score_steps(b, out, nc, q_in, k_in, attn_out, logp_out,
                 w_kw1, w_kw2, w_qw1, w_qw2, w_qw3,
                 b_k1, b_k2, b_q1, b_q2, b_q3,
                 ones128, k1pad, q1pad, q2pad,
                 p_in, p_k, p_q, p_soft, p_small, p_att, p_lgp,
                 pp_conv, pp_score):
    mm = nc.tensor.matmul
    act = nc.scalar.activation
    ksb, k2s, q3 = out["ksb"], out["k2s"], out["q3"]
    attn_g = logp_g = None
    for t in range(TQ // 128):
        g, i = divmod(t, 2)
        # the last 4 tiles borrow the (then-idle) conv PSUM pool: 4 score
        # tiles in flight deepens the final drain pipeline
        pool = pp_conv if (b == BPC - 1 and t >= 12) else pp_score
        sp = pool.tile([128, 2, 512], F32,
                       tag="sc" if pool is pp_score else "conv",
                       name=f"sp{b}_{t}")
        if QK_FP8:
            for sh in range(2):
                mm(sp[:, sh, :HT2], q3[:, :, 128 * t:128 * (t + 1)],
                   ksb[:, :, HT2 * sh:HT2 * sh + HT2],
                   start=True, stop=True, perf_mode=DR)
        else:
            for c in range(2):
                q3w = q3[:, c, 128 * t:128 * (t + 1)]
                for sh in range(2):
                    mm(sp[:, sh, :HT2], q3w,
                       ksb[:, c, HT2 * sh:HT2 * sh + HT2],
                       start=(c == 0), stop=(c == 1))

        # lraw = SC*sp + k2s  (single PSUM reader; frees the bank)
        lraw = p_soft.tile([128, 2, HT2], F16, tag="lraw", name=f"lr{b}_{t}")
        nc.vector.scalar_tensor_tensor(
            out=lraw[:, :, :], in0=sp[:, :, :HT2], scalar=float(SC),
            in1=k2s[:, :, :], op0=ALU.mult, op1=ALU.add)

        esb = p_soft.tile([128, 2, HT2], F16, tag="esb", name=f"esb{b}_{t}")
        z = p_small.tile([128, 1], F32, tag="z")
        act(esb[:, :, :], lraw[:, :, :], AF.Exp, bias=0.0, scale=1.0,
            accum_out=z)
        if i == 0:
            attn_g = p_att.tile([128, 2, TK2], F16, tag="attn")
            logp_g = p_lgp.tile([128, 2, TK2], F16, tag="logp")
        lnz = p_small.tile([128, 1], F32, tag="lnz")
        act(lnz, z, AF.Ln)
        rz = p_small.tile([128, 1], F32, tag="rz")
        nc.vector.reciprocal(rz, z)
        nc.vector.tensor_scalar(
            attn_g[:, i, :], esb.rearrange("p a b -> p (a b)"),
            rz, None, ALU.mult)
        nc.vector.tensor_scalar(
            logp_g[:, i, :], lraw.rearrange("p a b -> p (a b)"),
            lnz, None, ALU.subtract)
        if b == BPC - 1 and g == 7:
            # last group: per-tile DMA so the final transfer starts early
            nc.sync.dma_start(out=attn_out[b, g, :, i], in_=attn_g[:, i, :])
            nc.sync.dma_start(out=logp_out[b, g, :, i], in_=logp_g[:, i, :])
        elif i == 1:
            nc.sync.dma_start(out=attn_out[b, g], in_=attn_g[:, :, :])
            nc.sync.dma_start(out=logp_out[b, g], in_=logp_g[:, :, :])
        yield


_PROGRAM = None


def _get_program():
    global _PROGRAM
    if _PROGRAM is None:
        _PROGRAM = build_program()
    return _PROGRAM


def prep_inputs(queries, keys, kw1, kb1, kw2, kb2, qw1, qb1, qw2, qb2, qw3, qb3):
    """Build the 8 per-core input maps from full-size inputs.

    All tensors are laid out partition-major (first dim = SBUF partition)
    with contiguous per-partition payloads -> one large-burst DMA each."""
    f32 = np.float32
    n_8 = mybir.dt.np(DT8)
    n_q = mybir.dt.np(DT_Q)

    kw1t = np.ascontiguousarray(np.transpose(
        np.transpose(kw1, (1, 2, 0)).reshape(4, 128, 3, DH), (1, 0, 2, 3))
        .astype(n_8))
    kw2t = np.ascontiguousarray(np.transpose(
        np.transpose(kw2, (1, 2, 0)).reshape(2, 128, 3, DH), (1, 0, 2, 3))
        .astype(np.float16))
    # qc1 packed weights: row m = 80*j + c -> qw1[o, c, j]
    qw1t = np.zeros((5 * 128, DH), n_q)
    qw1t[:560] = np.transpose(qw1, (2, 1, 0)).reshape(560, DH)
    qw1t = np.ascontiguousarray(
        np.transpose(qw1t.reshape(5, 128, DH), (1, 0, 2)))
    qw2t = np.ascontiguousarray(np.transpose(
        np.transpose(qw2, (1, 2, 0)).reshape(2, 128, 7, DH), (1, 0, 2, 3))
        .astype(n_8))
    qw3t = np.ascontiguousarray(np.transpose(
        np.transpose(qw3, (1, 2, 0)).reshape(2, 128, 7, DH), (1, 0, 2, 3))
        .astype(n_8))
    bias10 = np.ascontiguousarray(np.stack(
        [b.reshape(2, 128) for b in (kb1, kb2, qb1, qb2, qb3)],
        axis=0).reshape(10, 128).T.astype(f32))
    shared = dict(kw1t=kw1t, kw2t=kw2t, qw1t=qw1t, qw2t=qw2t, qw3t=qw3t,
                  bias10=bias10)
    B = queries.shape[0]
    # packed queries: plane m = (j, c): qp[b, m, x] = queries[b, c, x + j - 3]
    qp = np.zeros((B, 5 * 128, TQ + 8), n_q)
    qd = queries.astype(n_q)
    for j in range(7):
        lo, hi = max(0, 3 - j), min(TQ, TQ + 3 - j)
        qp[:, 80 * j:80 * (j + 1), lo:hi] = qd[:, :, lo + j - 3:hi + j - 3]
    qp = np.transpose(qp.reshape(B, 5, 128, TQ + 8), (0, 2, 1, 3))
    kp = np.zeros((B, D_ENC, TK + 6), n_8)
    kp[:, :, 3:TK + 3] = keys.astype(n_8)
    kp = np.transpose(kp.reshape(B, 4, 128, TK + 6), (0, 2, 1, 3))
    in_maps = []
    for i in range(N_CORES):
        m = dict(shared)
        m["queries"] = np.ascontiguousarray(qp[BPC * i:BPC * (i + 1)])
        m["keys"] = np.ascontiguousarray(kp[BPC * i:BPC * (i + 1)])
        in_maps.append(m)
    return in_maps


def run(in_maps, **kwargs):
    nc = _get_program()
    return run_bass_kernel_spmd(nc, in_maps, core_ids=list(range(N_CORES)), **kwargs)


def kernel(queries, keys, kw1, kb1, kw2, kb2, qw1, qb1, qw2, qb2, qw3, qb3,
           **kwargs):
    in_maps = prep_inputs(queries, keys, kw1, kb1, kw2, kb2,
                          qw1, qb1, qw2, qb2, qw3, qb3)
    res = run(in_maps)
    # device layout [b, g, p, i, s] with t = 256*g + 128*i + p
    attn = np.concatenate([np.asarray(r["attn_out"], np.float32)
                           for r in res.results], axis=0)
    logp = np.concatenate([np.asarray(r["logp_out"], np.float32)
                           for r in res.results], axis=0)
    B = attn.shape[0]
    attn = np.transpose(attn, (0, 1, 3, 2, 4)).reshape(B, 1, TQ, TK2)
    logp = np.transpose(logp, (0, 1, 3, 2, 4)).reshape(B, 1, TQ, TK2)
    return attn, logp
